# revision 3
# baseline (speedup 1.0000x reference)
"""Trainium2 Bass kernel for nn_Conv2d_34522947125875.

Conv2d: x (256,256,256) * weight (256,256,3,3) + bias -> (256,256,256),
stride 1, pad 1, fp32.

Strategy: spatial sharding over H across 8 NeuronCores (32 output rows per
core, 34-row input slab with halo, zero-padded host-side). On each core the
conv is computed as 18 accumulated matmuls per output tile (2 c_in blocks of
128 x 9 kernel taps) with the moving operand an access-pattern view of the
padded input slab: free dims (2 rows, 256 cols) with row stride 258 -> N=512.
Matmuls run in float32r (full PE rate; ~1.6e-4 rel err vs ~2.3e-7 for fp32).
Bias is fused into the PSUM->SBUF copy on the scalar engine.
"""
import os
import sys

for _p in ("/opt/trn_rl_repo", "/root/.axon_site/_ro/trn_rl_repo"):
    if os.path.isdir(_p) and _p not in sys.path:
        sys.path.insert(0, _p)

import numpy as np

C_IN, C_OUT, K, H, W = 256, 256, 3, 256, 256
PAD = 1
N_CORES = 8
ROWS = H // N_CORES          # 32 output rows per core
SLAB = ROWS + 2 * PAD        # 34 input rows per core
WP = W + 2 * PAD             # 258 padded width
CB = C_IN // 128             # 2 c_in blocks
OB = C_OUT // 128            # 2 c_out blocks
TAPS = K * K                 # 9
PAIRS = ROWS // 2            # 16 row-pairs (N=512 per matmul)

_program_cache = {}


def _build_program(mm_dtype_name: str):
    import concourse.mybir as mybir
    from concourse import bacc
    from concourse.tile import TileContext

    mm_dt = getattr(mybir.dt, mm_dtype_name)

    nc = bacc.Bacc("TRN2", num_devices=N_CORES)
    xs = nc.declare_dram_parameter("xs", [C_IN, SLAB, WP], mm_dt, isOutput=False)
    wt = nc.declare_dram_parameter("wt", [CB, 128, TAPS, C_OUT], mm_dt, isOutput=False)
    bs = nc.declare_dram_parameter("bs", [OB, 128], mybir.dt.float32, isOutput=False)
    ys = nc.declare_dram_parameter("ys", [C_OUT, ROWS, W], mybir.dt.float32, isOutput=True)

    with TileContext(nc) as tc:
        with (
            tc.tile_pool(name="xp", bufs=1) as xp,
            tc.tile_pool(name="wp", bufs=1) as wp_pool,
            tc.tile_pool(name="bp", bufs=1) as bp,
            tc.tile_pool(name="pp", bufs=4, space="PSUM") as pp,
            tc.tile_pool(name="op", bufs=4) as op,
        ):
            # weights + bias first (small), then input slab in row chunks so
            # early matmuls can start before the whole slab lands
            wtiles = []
            for ci in range(CB):
                wtile = wp_pool.tile([128, TAPS, C_OUT], mm_dt, tag=f"w{ci}")
                nc.sync.dma_start(out=wtile, in_=wt[ci])
                wtiles.append(wtile)
            bias_t = bp.tile([128, OB], mybir.dt.float32, tag="bias")
            nc.sync.dma_start(out=bias_t, in_=bs[:].rearrange("b p -> p b"))

            xtiles = []
            row_chunks = [(0, 9), (9, 18), (18, 26), (26, SLAB)]
            for ci in range(CB):
                xt = xp.tile([128, SLAB, WP], mm_dt, tag=f"x{ci}")
                src = xs[ci * 128:(ci + 1) * 128]
                for r0, r1 in row_chunks:
                    nc.sync.dma_start(out=xt[:, r0:r1, :], in_=src[:, r0:r1, :])
                xtiles.append(xt)

            for j in range(PAIRS):
                for cb in range(OB):
                    psum = pp.tile([128, 2, W], mybir.dt.float32, tag="ps")
                    step = 0
                    for ci in range(CB):
                        for kh in range(K):
                            for kw in range(K):
                                rhs = xtiles[ci][:, 2 * j + kh: 2 * j + kh + 2, kw: kw + W]
                                lhsT = wtiles[ci][:, kh * K + kw, cb * 128:(cb + 1) * 128]
                                nc.tensor.matmul(
                                    psum, lhsT=lhsT, rhs=rhs,
                                    start=(step == 0), stop=(step == CB * TAPS - 1),
                                )
                                step += 1
                    ot = op.tile([128, 2, W], mybir.dt.float32, tag="ot")
                    nc.scalar.activation(
                        ot, psum, mybir.ActivationFunctionType.Identity,
                        bias=bias_t[:, cb: cb + 1],
                    )
                    nc.sync.dma_start(
                        out=ys[cb * 128:(cb + 1) * 128, 2 * j: 2 * j + 2, :], in_=ot
                    )

    nc.compile()
    return nc


def _ensure_ntff_hook() -> bool:
    """Register the axon NTFF profile hook if the image's antenv lacks it."""
    import types

    try:
        from antenv.axon_hooks import get_axon_ntff_profile_hook  # noqa: F401
        return True
    except ImportError:
        pass
    try:
        import antenv
        from trn_agent_boot.trn_boot import _ntff_profile_via_ctypes

        hook = _ntff_profile_via_ctypes("/opt/axon/libaxon_pjrt.so")
        if hook is None:
            return False
        mod = types.ModuleType("antenv.axon_hooks")
        mod._hook = hook
        mod.get_axon_ntff_profile_hook = lambda: mod._hook

        def _set(h):
            mod._hook = h

        mod.set_axon_ntff_profile_hook = _set
        sys.modules["antenv.axon_hooks"] = mod
        antenv.axon_hooks = mod
        return True
    except Exception:
        return False


def kernel(x: np.ndarray, weight: np.ndarray, bias: np.ndarray) -> np.ndarray:
    from concourse.bass_utils import run_bass_kernel_spmd

    mm_dtype = os.environ.get("CONV_MM_DTYPE", "float32r")
    trace = os.environ.get("CONV_TRACE", "0") == "1"
    if trace:
        trace = _ensure_ntff_hook()

    key = mm_dtype
    if key not in _program_cache:
        _program_cache[key] = _build_program(mm_dtype)
    nc = _program_cache[key]

    x = np.ascontiguousarray(x, dtype=np.float32)
    weight = np.ascontiguousarray(weight, dtype=np.float32)
    bias = np.ascontiguousarray(bias, dtype=np.float32).reshape(C_OUT)

    # zero-pad input spatially; slabs share halo rows
    x_pad = np.zeros((C_IN, H + 2 * PAD, WP), dtype=np.float32)
    x_pad[:, PAD:PAD + H, PAD:PAD + W] = x
    # weight -> lhsT layout [ci_blk][128 ci, tap, co]
    wl = np.ascontiguousarray(
        weight.transpose(1, 2, 3, 0).reshape(CB, 128, TAPS, C_OUT)
    )
    bias2 = np.ascontiguousarray(bias.reshape(OB, 128))

    in_maps = []
    for c in range(N_CORES):
        slab = np.ascontiguousarray(x_pad[:, c * ROWS: c * ROWS + SLAB, :])
        in_maps.append({"xs": slab, "wt": wl, "bs": bias2})

    res = run_bass_kernel_spmd(nc, in_maps, list(range(N_CORES)), trace=trace)
    if trace and res.exec_time_ns is not None:
        print(f"HW exec time: {res.exec_time_ns} ns")
        kernel.last_exec_time_ns = res.exec_time_ns
        kernel.last_results = res

    out = np.empty((C_OUT, H, W), dtype=np.float32)
    for c in range(N_CORES):
        out[:, c * ROWS:(c + 1) * ROWS, :] = res.results[c]["ys"]
    return out


if __name__ == "__main__":
    rng = np.random.default_rng(0)
    x = rng.standard_normal((C_IN, H, W), dtype=np.float32)
    w = rng.standard_normal((C_OUT, C_IN, K, K), dtype=np.float32) * 0.02
    b = rng.standard_normal((C_OUT,), dtype=np.float32).reshape(C_OUT, 1, 1)
    y = kernel(x=x, weight=w, bias=b)
    print("out", y.shape, y.dtype, float(np.abs(y).max()))
